# revision 58
# baseline (speedup 1.0000x reference)
"""Multi-head causal attention (B=2, S=2048, H=1024, 16 heads x 64, RoPE) on 8 trn2 cores.

Sharding: tensor-parallel over heads (2 heads/core) for QKV+attention, then
AllToAlls switch to token-parallel for the output projection. Each core owns
two 128-token slices per batch; the host reassembles the full output.

Design (per core c, heads h0=2c, h0+1):
 - xT [1024, 4096] feature-major bf16 activations (host-transposed), so qT/kT
   (feature-major) come straight out of matmuls, and V comes out token-major
   by using the x-chunk as the stationary operand (no PE transposes at all).
 - RoPE: q/k weight ROWS are host-permuted to [0:16,32:48,16:32,48:64] per
   head so the rotate-half partner lives in the same 32-partition quadrant;
   the partition swap is then a single DVE stream_shuffle (no DMAs). cos
   multiply fused with the PSUM->SBUF copy on DVE, sin multiply on GpSimd,
   add on DVE. qT/kT stored bf16 (full PE rate at any matmul width).
 - Scores TRANSPOSED: sT[k, q] = matmul(lhsT=kT_block, rhs=qT_chunk), both
   heads side by side in one [128, 1024] PSUM tile (2 banks) -> ONE exp
   instruction per key-block covers both heads (halves ACT instruction
   overhead). Max-subtraction skipped (logits ~N(0,1)). Causal mask = bf16
   0/1 multiply on the diagonal block's probs (both heads in one DVE op).
 - V carries a ones-column (65-wide lhsT): the PV matmul emits the softmax
   denominators as PSUM row 64 for free - no separate selector matmuls.
 - Normalization per head: DVE reciprocal (bf16) of the sums row, K=1 bf16
   broadcast matmul, then the PSUM->SBUF context copy fused with the divide.
 - Fully pipelined schedule: attention blocks run in ASCENDING query order,
   interleaved with the QKV chunk that unlocks the next block, so the first
   AllToAll piece launches ~30us in. The ctx handoff is 4 AllToAlls of
   [8,128,128] bf16 (one per 1024-token half per batch), each launched as
   soon as its half is done; out-proj halves run as their collective lands,
   and a throwaway-matmul chain keeps the PE pstate hot across the last
   exposed collective. Received buffers are exactly ctx_shard.T = lhsT of
   the out-proj (x W_out.T, fp32 out).
"""

import numpy as np

import concourse.bacc as bacc
import concourse.mybir as mybir
import concourse.tile as tile
from concourse.bass_utils import run_bass_kernel_spmd

F32 = mybir.dt.float32
BF16 = mybir.dt.bfloat16
EXP = mybir.ActivationFunctionType.Exp

B, S, H = 2, 2048, 1024
NH, HD = 16, 64
NCORES = 8
T = B * S            # 4096 flattened tokens (b-major)
P = 128

# rotate-half partner lives 16 partitions away inside each 32-quadrant
SWAP_MASK = list(range(16, 32)) + list(range(0, 16))


def _build_nc():
    nc = bacc.Bacc(None, num_devices=NCORES)

    xT_d = nc.dram_tensor("xT", [H, T], BF16, kind="ExternalInput")
    wqkvT_d = nc.dram_tensor("wqkvT", [H, 384], BF16, kind="ExternalInput")
    woutT_d = nc.dram_tensor("woutT", [H, H], BF16, kind="ExternalInput")
    costab_d = nc.dram_tensor("costab", [P, S], BF16, kind="ExternalInput")
    sintab_d = nc.dram_tensor("sintab", [P, S], BF16, kind="ExternalInput")
    maskT2_d = nc.dram_tensor("maskT2", [P, 256], BF16, kind="ExternalInput")
    onesel_d = nc.dram_tensor("onesel", [P, 64], BF16, kind="ExternalInput")
    out_d = nc.dram_tensor("out", [2 * 256, H], F32, kind="ExternalOutput")

    with tile.TileContext(nc) as tc:
        with (
            tc.tile_pool(name="long", bufs=1) as lp,
            tc.tile_pool(name="dram", bufs=1, space="DRAM") as dp,
            tc.tile_pool(name="px", bufs=3) as px,
            tc.tile_pool(name="pswp", bufs=4) as pswp,
            tc.tile_pool(name="ppb", bufs=10) as ppb,
            tc.tile_pool(name="prb", bufs=3) as prb,
            tc.tile_pool(name="pob", bufs=3) as pob,
            tc.tile_pool(name="psA", bufs=2, space="PSUM") as psA,
            tc.tile_pool(name="psB", bufs=2, space="PSUM") as psB,
            tc.tile_pool(name="psC", bufs=1, space="PSUM") as psC,
        ):
            qT = [lp.tile([P, S], BF16, tag=f"qT{b}", name=f"qT{b}")
                  for b in range(B)]
            kT = [lp.tile([P, S], BF16, tag=f"kT{b}", name=f"kT{b}")
                  for b in range(B)]
            # V: per token-block gb, [h0 d0..63, ones | h1 d0..63, ones]
            V = [lp.tile([P, 16, 130], BF16, tag=f"V{b}", name=f"V{b}")
                 for b in range(B)]
            ctxT = [lp.tile([P, S], BF16, tag=f"ctxT{b}", name=f"ctxT{b}")
                    for b in range(B)]
            maskT2_t = lp.tile([P, 256], BF16, tag="maskT2")
            onesel_t = lp.tile([P, 64], BF16, tag="onesel")
            wq = lp.tile([P, 8, 384], BF16, tag="wq")
            wo = lp.tile([P, 8, H], BF16, tag="wo")
            costab_t = lp.tile([P, S], BF16, tag="costab")
            sintab_t = lp.tile([P, S], BF16, tag="sintab")

            a2aA_in = [dp.tile([NCORES, P, P], BF16, name=f"a2aA_in{b}",
                               tag=f"a2aA_in{b}") for b in range(B)]
            a2aA_out = [dp.tile([NCORES, P, P], BF16, name=f"a2aA_out{b}",
                                tag=f"a2aA_out{b}") for b in range(B)]
            a2aB_in = [dp.tile([NCORES, P, P], BF16, name=f"a2aB_in{b}",
                               tag=f"a2aB_in{b}") for b in range(B)]
            a2aB_out = [dp.tile([NCORES, P, P], BF16, name=f"a2aB_out{b}",
                                tag=f"a2aB_out{b}") for b in range(B)]

            wqkv_r = wqkvT_d[:].rearrange("(k p) c -> p k c", p=P)
            nc.sync.dma_start(wq[:, 0:2, :], wqkv_r[:, 0:2, :])
            # ones columns of V (cols 64 and 129 of every token block)
            for b in range(B):
                vsel = V[b][:].rearrange("p g (s c) -> p g s c", s=2, c=65)
                nc.vector.memset(vsel[:, :, :, 64:65], 1.0)

            state = {"b1ch": 0, "pw": 0}

            def chunk_qkv(b, ch):
                tok0 = b * S + ch * 512
                c0 = ch * 512
                xt_r = (xT_d[:, tok0:tok0 + 512]
                        .rearrange("(k p) t -> p k t", p=P))
                xta = px.tile([P, 4, 512], BF16, tag="xta")
                xtb = px.tile([P, 4, 512], BF16, tag="xtb")
                if b == 0 and ch == 0:
                    # dependency-ordered: kt0-1 ready after 2 transfers
                    nc.sync.dma_start(xta[:, 0:2, :], xt_r[:, 0:2, :])
                    nc.sync.dma_start(wq[:, 2:4, :], wqkv_r[:, 2:4, :])
                    nc.sync.dma_start(xta[:, 2:4, :], xt_r[:, 2:4, :])
                    nc.sync.dma_start(wq[:, 4:6, :], wqkv_r[:, 4:6, :])
                    nc.sync.dma_start(xtb[:, 0:2, :], xt_r[:, 4:6, :])
                    nc.sync.dma_start(wq[:, 6:8, :], wqkv_r[:, 6:8, :])
                    nc.sync.dma_start(xtb[:, 2:4, :], xt_r[:, 6:8, :])
                else:
                    nc.sync.dma_start(xta[:], xt_r[:, 0:4, :])
                    nc.sync.dma_start(xtb[:], xt_r[:, 4:8, :])
                if b == 0 and ch == 0:
                    nc.sync.dma_start(costab_t[:], costab_d[:])
                    nc.sync.dma_start(sintab_t[:], sintab_d[:])
                    nc.sync.dma_start(maskT2_t[:], maskT2_d[:])
                    nc.sync.dma_start(onesel_t[:], onesel_d[:])
                w = slice(c0, c0 + 512)
                pss = []
                for m in range(2):      # q, k
                    ps = psA.tile([P, 512], F32, tag="qkv")
                    for kt in range(8):
                        xt_half = xta if kt < 4 else xtb
                        nc.tensor.matmul(
                            ps[:],
                            wq[:, kt, m * P:(m + 1) * P],
                            xt_half[:, kt % 4, :],
                            start=(kt == 0), stop=(kt == 7),
                        )
                    pss.append(ps)
                swps = []
                for m in range(2):      # rope: shuffle + cos (DVE), sin (Pool)
                    tgt = (qT[b] if m == 0 else kT[b])[:, w]
                    swp = pswp.tile([P, 512], F32, tag="swp")
                    swpb = pswp.tile([P, 512], BF16, tag="swpb")
                    nc.vector.stream_shuffle(swp[:], pss[m][:], SWAP_MASK)
                    nc.vector.tensor_mul(tgt, pss[m][:], costab_t[:, w])
                    nc.gpsimd.tensor_mul(swpb[:], swp[:], sintab_t[:, w])
                    swps.append(swpb)
                for m in range(2):
                    tgt = (qT[b] if m == 0 else kT[b])[:, w]
                    nc.vector.tensor_add(tgt, tgt, swps[m][:])
                for tb in range(4):     # v, token-major directly
                    ps = psA.tile([P, 512], F32, tag="qkv")
                    for kt in range(8):
                        xt_half = xta if kt < 4 else xtb
                        nc.tensor.matmul(
                            ps[:, 0:128],
                            xt_half[:, kt % 4, tb * P:(tb + 1) * P],
                            wq[:, kt, 256:384],
                            start=(kt == 0), stop=(kt == 7),
                        )
                    gb = ch * 4 + tb
                    vdst = (V[b][:, gb, 0:130]
                            .rearrange("p (s c) -> p s c", s=2, c=65))
                    vsrc = (ps[:, 0:128]
                            .rearrange("p (s c) -> p s c", s=2, c=64))
                    nc.scalar.copy(vdst[:, :, 0:64], vsrc[:])

            def pv_emit(b, qs, nkb, pctx, item):
                kb, pb, qoff, N = item
                for h in range(2):
                    nc.tensor.matmul(
                        pctx[0:65, h * 512 + qoff:h * 512 + 512],
                        V[b][:, kb, 65 * h:65 * h + 65],
                        pb[:, h * 512 + qoff:h * 512 + 512],
                        start=(kb == 0), stop=(kb == nkb - 1),
                        skip_group_check=True,
                    )

            def att_body(b, qs):
                pctx = psC.tile([P, 1024], F32, tag="ctx")
                nkb = 4 * qs + 4
                pend = []
                for kb in range(nkb):
                    j = kb - 4 * qs
                    qoff = max(0, j) * P
                    N = 512 - qoff
                    psT = psB.tile([P, 1024], F32, tag="sT")
                    for h in range(2):
                        nc.tensor.matmul(
                            psT[:, h * 512 + qoff:h * 512 + 512],
                            kT[b][h * HD:(h + 1) * HD,
                                  kb * P:(kb + 1) * P],
                            qT[b][h * HD:(h + 1) * HD,
                                  qs * 512 + qoff:(qs + 1) * 512],
                            start=True, stop=True,
                            tile_position=(h * HD, 0),
                            skip_group_check=True,
                        )
                    pb = ppb.tile([P, 1024], BF16, tag="pb")
                    psT_r = psT[:].rearrange("p (h q) -> p h q", h=2)
                    pb_r = pb[:].rearrange("p (h q) -> p h q", h=2)
                    nc.scalar.activation(pb_r[:, :, qoff:512],
                                         psT_r[:, :, qoff:512],
                                         EXP, scale=0.125)
                    if j >= 0:
                        mr = maskT2_t[:].rearrange("p (h q) -> p h q", h=2)
                        nc.vector.tensor_mul(pb_r[:, :, qoff:qoff + P],
                                             pb_r[:, :, qoff:qoff + P],
                                             mr[:])
                    pend.append((kb, pb, qoff, N))
                    while len(pend) > 2:
                        pv_emit(b, qs, nkb, pctx, pend.pop(0))
                while pend:
                    pv_emit(b, qs, nkb, pctx, pend.pop(0))
                return pctx

            def att_finish(b, qs, pctx):
                # normalize, pipelined per head so pctx frees sooner
                w = slice(qs * 512, (qs + 1) * 512)
                rbf = prb.tile([P, 1024], BF16, tag="rbf")
                pbc = psB.tile([P, 1024], F32, tag="sT")
                rbb = prb.tile([P, 512], F32, tag="rbb")
                for h in range(2):
                    hw = slice(h * 512, (h + 1) * 512)
                    with nc.allow_low_precision("softmax denom in bf16"):
                        nc.vector.reciprocal(rbf[64:65, hw], pctx[64:65, hw])
                    nc.tensor.matmul(pbc[h * 64:(h + 1) * 64, 0:512],
                                     onesel_t[64:65, :], rbf[64:65, hw],
                                     start=True, stop=True,
                                     skip_group_check=True)
                    nc.scalar.copy(rbb[h * 64:(h + 1) * 64, :],
                                   pbc[h * 64:(h + 1) * 64, 0:512])
                    nc.vector.tensor_mul(ctxT[b][h * 64:(h + 1) * 64, w],
                                         pctx[0:64, hw],
                                         rbb[h * 64:(h + 1) * 64, :])
                dst = (a2aA_in[b] if qs >= 2 else a2aB_in[b])
                j0 = 4 * (qs % 2)
                nc.gpsimd.dma_start(
                    dst[j0:j0 + 4].rearrange("j p c -> p j c"),
                    ctxT[b][:, w].rearrange("p (j c) -> p j c", j=4))

            def op_loadh(bb, ctxs, half):
                t = a2aA_out[bb] if half == 0 else a2aB_out[bb]
                nc.sync.dma_start(
                    ctxs[:, :, half * P:(half + 1) * P],
                    t[:].rearrange("j p c -> p j c"))

            def op_unit(bb, ctxs, obs, mt, nt):
                po = psB.tile([P, 1024], F32, tag="sT")
                for jj in range(8):
                    nc.tensor.matmul(
                        po[:, 0:512],
                        ctxs[:, jj, mt * P:(mt + 1) * P],
                        wo[:, jj, nt * 512:(nt + 1) * 512],
                        start=(jj == 0), stop=(jj == 7),
                        skip_group_check=True,
                    )
                ob = pob.tile([P, 512], F32, tag="ob",
                              name=f"ob{bb}{mt}{nt}")
                nc.vector.tensor_copy(ob[:], po[:, 0:512])
                nc.sync.dma_start(
                    out_d[bb * 256 + mt * P:bb * 256 + (mt + 1) * P,
                          nt * 512:(nt + 1) * 512],
                    ob[:])

            def op_unit_split(bb, ctxs, mt, nt):
                # final unit: two 256-col groups so the first store hides
                # under the second group's matmuls
                for g in range(2):
                    po = psB.tile([P, 1024], F32, tag="sT")
                    n0 = nt * 512 + g * 256
                    for jj in range(8):
                        nc.tensor.matmul(
                            po[:, 0:256],
                            ctxs[:, jj, mt * P:(mt + 1) * P],
                            wo[:, jj, n0:n0 + 256],
                            start=(jj == 0), stop=(jj == 7),
                            skip_group_check=True,
                        )
                    ob = pob.tile([P, 256], F32, tag="obs",
                                  name=f"obs{bb}{mt}{nt}{g}")
                    nc.vector.tensor_copy(ob[:], po[:, 0:256])
                    nc.sync.dma_start(
                        out_d[bb * 256 + mt * P:bb * 256 + (mt + 1) * P,
                              n0:n0 + 256],
                        ob[:])

            def pe_warm(n):
                # keep the PE pstate hot across the exposed collective with
                # throwaway matmuls (results never read)
                for i in range(n):
                    state["pw"] += 1
                    pw = psA.tile([P, 512], F32, tag="qkv",
                                  name=f"pw{state['pw']}")
                    nc.tensor.matmul(pw[:], wo[:, 0, 0:P],
                                     ctxT[1][:, 0:512],
                                     start=True, stop=True,
                                     skip_group_check=True)

            def collective_split(tin, tout):
                nc.gpsimd.collective_compute(
                    "AllToAll",
                    mybir.AluOpType.bypass,
                    replica_groups=[list(range(NCORES))],
                    ins=[tin.opt()],
                    outs=[tout.opt()],
                )

            # ---------------- schedule
            chunk_qkv(0, 0)
            pctx = att_body(0, 0)
            chunk_qkv(0, 1)
            att_finish(0, 0, pctx)
            pctx = att_body(0, 1)
            chunk_qkv(0, 2)
            att_finish(0, 1, pctx)
            collective_split(a2aB_in[0], a2aB_out[0])
            pctx = att_body(0, 2)
            chunk_qkv(0, 3)
            nc.sync.dma_start(
                wo[:], woutT_d[:].rearrange("(j p) n -> p j n", p=P))
            att_finish(0, 2, pctx)
            ctxs0 = pob.tile([P, 8, 256], BF16, tag="ctxs",
                             name="ctxs0", bufs=1)
            ctxs1 = pob.tile([P, 8, 256], BF16, tag="ctxs",
                             name="ctxs1", bufs=1)
            obs0 = {}
            obs1 = {}
            pctx = att_body(0, 3)
            chunk_qkv(1, 0)
            op_loadh(0, ctxs0, 1)
            op_unit(0, ctxs0, obs0, 1, 0)
            op_unit(0, ctxs0, obs0, 1, 1)
            att_finish(0, 3, pctx)
            collective_split(a2aA_in[0], a2aA_out[0])
            chunk_qkv(1, 1)
            pctx = att_body(1, 0)
            chunk_qkv(1, 2)
            att_finish(1, 0, pctx)
            pctx = att_body(1, 1)
            chunk_qkv(1, 3)
            att_finish(1, 1, pctx)
            collective_split(a2aB_in[1], a2aB_out[1])
            pctx = att_body(1, 2)
            att_finish(1, 2, pctx)
            pctx = att_body(1, 3)
            op_loadh(0, ctxs0, 0)
            op_unit(0, ctxs0, obs0, 0, 0)
            op_unit(0, ctxs0, obs0, 0, 1)
            att_finish(1, 3, pctx)
            collective_split(a2aA_in[1], a2aA_out[1])
            op_loadh(1, ctxs1, 1)
            op_unit(1, ctxs1, obs1, 1, 0)
            op_unit(1, ctxs1, obs1, 1, 1)
            pe_warm(185)
            srcA = a2aA_out[1][:].rearrange("j p c -> p j c")
            nc.sync.dma_start(ctxs1[:, 0:4, 0:P], srcA[:, 0:4, :])
            nc.gpsimd.dma_start(ctxs1[:, 4:8, 0:P], srcA[:, 4:8, :])
            op_unit(1, ctxs1, obs1, 0, 0)
            op_unit_split(1, ctxs1, 0, 1)

    nc.finalize()
    return nc


_NC_CACHE = None


def _get_nc():
    global _NC_CACHE
    if _NC_CACHE is None:
        _NC_CACHE = _build_nc()
    return _NC_CACHE


# original rope dim -> stored row (per 64-dim head): [0:16, 32:48, 16:32, 48:64]
_ROPE_PERM = np.concatenate([
    np.arange(0, 16), np.arange(32, 48),
    np.arange(16, 32), np.arange(48, 64),
])


def _host_tables():
    import ml_dtypes
    j = np.arange(32)
    inv = (10000.0 ** (-(j.astype(np.float64)) / 32.0))
    pos = np.arange(S, dtype=np.float64)
    fr = pos[:, None] * inv[None, :]              # [S, 32]
    cosT = np.cos(fr).T.astype(np.float32)        # [32, S]
    sinT = np.sin(fr).T.astype(np.float32)
    # stored row r holds original dim d = _ROPE_PERM[r]; freq j = d mod 32,
    # sin sign = -1 for d < 32 (first half), +1 for d >= 32
    cos64 = np.empty((64, S), np.float32)
    sin64 = np.empty((64, S), np.float32)
    for r, d in enumerate(_ROPE_PERM):
        jj = d % 32
        cos64[r] = cosT[jj]
        sin64[r] = sinT[jj] if d >= 32 else -sinT[jj]
    costab = np.tile(cos64, (2, 1)).astype(ml_dtypes.bfloat16)   # [128, S]
    sintab = np.tile(sin64, (2, 1)).astype(ml_dtypes.bfloat16)
    kk = np.arange(P)[:, None]
    qq = np.arange(P)[None, :]
    m = np.where(kk <= qq, 1.0, 0.0).astype(ml_dtypes.bfloat16)
    maskT2 = np.concatenate([m, m], axis=1)       # [128, 256]
    onesel = np.ones((P, 64), dtype=np.float32).astype(ml_dtypes.bfloat16)
    return costab, sintab, maskT2, onesel


def _make_in_maps(x, W_qkv, W_out):
    import ml_dtypes
    costab, sintab, maskT2, onesel = _host_tables()
    xT = np.ascontiguousarray(x.reshape(T, H).T).astype(ml_dtypes.bfloat16)
    woutT = np.ascontiguousarray(W_out.T).astype(ml_dtypes.bfloat16)
    in_maps = []
    for c in range(NCORES):
        h0 = 2 * c
        # q/k weight rows permuted so rope partners share a 32-quadrant
        qrows = W_qkv[HD * h0:HD * (h0 + 2)]
        krows = W_qkv[H + HD * h0:H + HD * (h0 + 2)]
        vrows = W_qkv[2 * H + HD * h0:2 * H + HD * (h0 + 2)]
        perm128 = np.concatenate([_ROPE_PERM, 64 + _ROPE_PERM])
        rows = np.concatenate([qrows[perm128], krows[perm128], vrows], axis=0)
        wqkvT = np.ascontiguousarray(rows.T).astype(ml_dtypes.bfloat16)
        in_maps.append({
            "xT": xT, "wqkvT": wqkvT, "woutT": woutT,
            "costab": costab, "sintab": sintab,
            "maskT2": maskT2, "onesel": onesel,
        })
    return in_maps


def _run_spmd(x, W_qkv, W_out, **kw):
    nc = _get_nc()
    in_maps = _make_in_maps(x, W_qkv, W_out)
    return run_bass_kernel_spmd(nc, in_maps, core_ids=list(range(NCORES)),
                                **kw)


def kernel(x, W_qkv, W_out):
    x = np.asarray(x, dtype=np.float32)
    W_qkv = np.asarray(W_qkv, dtype=np.float32)
    W_out = np.asarray(W_out, dtype=np.float32)
    res = _run_spmd(x, W_qkv, W_out)
    # core c owns tokens [c*256,(c+1)*256) of each batch (flattened b-major)
    full = np.empty((T, H), dtype=np.float32)
    for c in range(NCORES):
        o = res.results[c]["out"]
        # per batch: rows +0:128 = A tokens 1024+c*128, +128:256 = B c*128
        for b in range(B):
            full[b * S + 1024 + c * P:b * S + 1024 + (c + 1) * P] = \
                o[b * 256:b * 256 + P]
            full[b * S + c * P:b * S + (c + 1) * P] = \
                o[b * 256 + P:b * 256 + 256]
    return full.reshape(B, S, H)
